# revision 16
# baseline (speedup 1.0000x reference)
"""Multi-head causal attention (B=2, S=2048, D=1024, H=16) on 8 TRN2 NeuronCores.

Sharding: batch*head parallel. Core c handles batch b = c//4 and the 4
heads h in [4*(c%4), 4*(c%4)+4). Each core computes its heads' Q/K/V
projections (column-parallel), causal softmax attention, and its partial
row-parallel output projection; the host sums the 4 partial outputs per
batch (the AllReduce of row-parallel tensor parallelism).

On-device layout: everything is kept "transposed" (feature-major) so
every matmul contracts along the partition dimension:
  scoresT[k,q] = K Q^T      (per head, 128-row k-tiles x 512-col q-tiles)
  P^T = exp(scoresT/8 + mask/8)   (additive -1e9 causal mask, PE-accumulated)
  outT[d,q]   = sum_k V[k,d] P^T[k,q]   (PSUM-accumulated over k-tiles)
  sums[q]     = sum_k P^T[k,q]          (ones-vector matmul, col-packed)
  y[q,e]     += sum_hd outT_norm[hd,q] * w_oT[hd,e]
Softmax skips the max-subtraction: scores ~ N(0,1), so exp never
overflows fp32, and exp(-1e9/8) underflows to exactly 0 like the
reference's masked_fill(-1e9).

Matmuls run as float32r (TF32-like, 1 cycle/row at N>=512; measured
~1.5e-4 rms per matmul). Fully-masked 128x512 blocks are skipped
(causal => ~62% of blocks computed).
"""

import numpy as np

D_MODEL = 1024
N_HEADS = 16
D_K = 64
B, S = 2, 2048
N_CORES = 8
HPC = 4            # heads per core
KT = S // 128      # 16 k-tiles
QT = S // 512      # 4 q-tiles
ET = D_MODEL // 128  # 8 e-tiles (contraction tiles for projections)

ATT_BF16 = False  # bf16 scores/attnV matmuls (f32r projections + output proj)

_PROG_CACHE = {}


def _build_program():
    import concourse.bacc as bacc_mod
    import concourse.mybir as mybir
    import concourse.tile as tile

    f32 = mybir.dt.float32
    f32r = mybir.dt.float32r
    bf16 = mybir.dt.bfloat16
    att_dt = bf16 if ATT_BF16 else f32r
    Exp = mybir.ActivationFunctionType.Exp

    nc = bacc_mod.Bacc(
        "TRN2", target_bir_lowering=False, debug=False, num_devices=N_CORES
    )

    xq = nc.dram_tensor("xq", [D_MODEL, S], f32r, kind="ExternalInput").ap()
    xk = nc.dram_tensor("xk", [D_MODEL, S], f32r, kind="ExternalInput").ap()
    xv = nc.dram_tensor("xv", [D_MODEL, S], f32r, kind="ExternalInput").ap()
    wq = nc.dram_tensor("wq", [D_MODEL, 256], f32r, kind="ExternalInput").ap()
    wk = nc.dram_tensor("wk", [D_MODEL, 256], f32r, kind="ExternalInput").ap()
    wv = nc.dram_tensor("wv", [D_MODEL, 256], f32r, kind="ExternalInput").ap()
    wo = nc.dram_tensor("wo", [256, D_MODEL], f32r, kind="ExternalInput").ap()
    maskt = nc.dram_tensor("maskt", [128, 2048], mybir.dt.bfloat16, kind="ExternalInput").ap()
    idbf = nc.dram_tensor("idbf", [128, 132], mybir.dt.bfloat16, kind="ExternalInput").ap()
    consts = nc.dram_tensor("consts", [128, 193], f32r, kind="ExternalInput").ap()
    y = nc.dram_tensor("y", [S, D_MODEL], f32, kind="ExternalOutput").ap()

    with (
        tile.TileContext(nc) as tc,
        nc.allow_low_precision("fp32r attention"),
        tc.tile_pool(name="persist", bufs=1) as pp,
    ):
        # ---- persistent SBUF tiles ----
        def persist(shape, dtype, name):
            return pp.tile(shape, dtype, name=name, tag=name)

        wq_sb = persist([128, ET * 256], f32r, "wq_sb")
        wk_sb = persist([128, ET * 256], f32r, "wk_sb")
        wv_sb = persist([128, ET * 256], f32r, "wv_sb")
        wo_sb = [persist([128, D_MODEL], f32r, f"wo_sb{p}") for p in range(2)]
        maskt_sb = persist([128, 2048], mybir.dt.bfloat16, "maskt_sb")
        idbf_sb = persist([128, 132], mybir.dt.bfloat16, "idbf_sb")
        consts_sb = persist([128, 193], f32r, "consts_sb")
        qt_sb = [persist([128, S], att_dt, f"qt_sb{p}") for p in range(2)]
        kt_sb = [persist([128, S], att_dt, f"kt_sb{p}") for p in range(2)]
        v_sb = [persist([128, 260], att_dt, f"v_sb{i}") for i in range(KT)]
        outt_sb = [persist([128, S], f32r, f"outt_sb{p}") for p in range(2)]

        identity = consts_sb[:, 0:128]
        ones_col = consts_sb[:, 128:192]   # [128, 64] of 1.0
        ones1 = consts_sb[:, 192:193]      # [128, 1] of 1.0

        # weight loads: [1024, 256] -> [128, 8*256] (e-tile t at cols 256t)
        for w_dram, w_tile in ((wq, wq_sb), (wk, wk_sb), (wv, wv_sb)):
            nc.sync.dma_start(
                out=w_tile[:].rearrange("p (t d) -> p t d", t=ET),
                in_=w_dram.rearrange("(t p) d -> p t d", p=128),
            )
        for p in range(2):
            nc.sync.dma_start(out=wo_sb[p][:], in_=wo[p * 128 : (p + 1) * 128, :])
        nc.sync.dma_start(out=maskt_sb[:], in_=maskt[:])
        nc.sync.dma_start(out=idbf_sb[:], in_=idbf[:])
        nc.sync.dma_start(out=consts_sb[:], in_=consts[:])

        # ---- PE warm-up ----
        # The PE HAM clock gate starts (and re-enters) K=4/8 half-clock and
        # only returns to full clock after ~3.4us of gapless PE activity.
        # Dense same-stationary dummy matmuls (results never read) force the
        # transition; packs are re-issued wherever the schedule has an
        # unavoidable multi-us PE idle (DMA-bound ramp, phase boundaries,
        # softmax-normalize tails).
        def emit_warm_pack(pool, count, tag="warm", name="warm"):
            wt = pool.tile([128, 512], f32, name=name, tag=tag)
            for w in range(count):
                nc.tensor.matmul(
                    wt[:],
                    idbf_sb[:, 0:128],
                    maskt_sb[:, 0:512],
                    start=True,
                    stop=True,
                )

        with tc.tile_pool(name="psW", bufs=1, space="PSUM") as psW:
            emit_warm_pack(psW, 16, name="warm_start")

        # ---- phase B1: Q^T and K^T projections ----
        # psum [128, 2048] per m-tile; accumulate over 8 e-tiles
        with (
            tc.tile_pool(name="xe", bufs=3) as xep,
            tc.tile_pool(name="psA", bufs=2, space="PSUM") as psA,
        ):
            for x_dram, w_tile, dst in ((xq, wq_sb, qt_sb), (xk, wk_sb, kt_sb)):
                ps = []
                for m in range(2):
                    ps.append(psA.tile([128, S], f32, name=f"ps_proj_{m}", tag="proj"))
                for e in range(ET):
                    xe = xep.tile([128, S], f32r, name=f"xe_{e}", tag="xe")
                    nc.sync.dma_start(out=xe[:], in_=x_dram[e * 128 : (e + 1) * 128, :])
                    for m in range(2):
                        lhsT = w_tile[:, e * 256 + m * 128 : e * 256 + (m + 1) * 128]
                        for n in range(QT):
                            nc.tensor.matmul(
                                ps[m][:, n * 512 : (n + 1) * 512],
                                lhsT,
                                xe[:, n * 512 : (n + 1) * 512],
                                start=(e == 0),
                                stop=(e == ET - 1),
                            )
                for m in range(2):
                    nc.vector.tensor_copy(dst[m][:], ps[m][:])

        # ---- phase B2: V projection (natural [k, d] layout) ----
        with (
            tc.tile_pool(name="xvk", bufs=2) as xvkp,
            tc.tile_pool(name="psV", bufs=2, space="PSUM") as psV,
        ):
            emit_warm_pack(psV, 18, tag="v", name="warm_v")
            for i in range(KT):
                xvk = xvkp.tile([128, ET * 128], f32r, name=f"xvk_{i}", tag="xvk")
                # columns 128i..128i+128 of xv as [128, 8*128]
                nc.sync.dma_start(
                    out=xvk[:].rearrange("p (t k) -> p t k", t=ET),
                    in_=xv[:, i * 128 : (i + 1) * 128].rearrange(
                        "(t p) k -> p t k", p=128
                    ),
                )
                psv = psV.tile([128, 256], f32, name=f"psv_{i}", tag="v")
                for e in range(ET):
                    nc.tensor.matmul(
                        psv[:],
                        xvk[:, e * 128 : (e + 1) * 128],
                        wv_sb[:, e * 256 : (e + 1) * 256],
                        start=(e == 0),
                        stop=(e == ET - 1),
                    )
                nc.vector.tensor_copy(
                    v_sb[i][:].rearrange("p (h c) -> p h c", c=65)[:, :, 0:64],
                    psv[:].rearrange("p (h d) -> p h d", d=64),
                )
                ones4 = idbf_sb[:, 128:132] if ATT_BF16 else consts_sb[:, 128:132]
                nc.vector.tensor_copy(
                    v_sb[i][:].rearrange("p (h c) -> p h c", c=65)[:, :, 64:65],
                    ones4.rearrange("p (h c) -> p h c", c=1),
                )

        # ---- phase C+D: attention with interleaved output projection ----
        # One head-pair per pass (pr = 0, 1). Per (pr, j): score tiles are
        # [128, 1024] head-pair PSUM tiles (row-packed score MMs fill the two
        # banks concurrently; ONE exp per round at FD=1024 runs ~2x faster
        # per element). attnV accumulates into a [65, 1024] pair tile (row
        # 64 = sum of exp via the ones column of v_sb). Normalization of
        # q-block j-1 is emitted lazily inside block j so its DVE chain and
        # broadcast matmuls never stall the PE; the output projection of
        # block j-1 runs as dense filler inside the pr=1 pass.
        with (
            tc.tile_pool(name="psS", bufs=2, space="PSUM") as psS,
            tc.tile_pool(name="psO", bufs=2, space="PSUM") as psO,
            tc.tile_pool(name="et", bufs=6) as etp,
            tc.tile_pool(name="bcsb", bufs=3) as bcp,
            tc.tile_pool(name="rcsb", bufs=3) as rcp,
            tc.tile_pool(name="ysb", bufs=3) as ysbp,
        ):
            def emit_outproj_mtile(m):
                psy = psS.tile([128, 1024], f32, name=f"psy_{m}", tag="s")
                for p in range(2):
                    for n in range(2):
                        nc.tensor.matmul(
                            psy[:, n * 512 : (n + 1) * 512],
                            outt_sb[p][:, m * 128 : (m + 1) * 128],
                            wo_sb[p][:, n * 512 : (n + 1) * 512],
                            start=(p == 0),
                            stop=(p == 1),
                        )
                y_sb = ysbp.tile([128, 1024], f32, name=f"y_sb_{m}", tag="ysb")
                nc.vector.tensor_copy(y_sb[:], psy[:])
                nc.sync.dma_start(out=y[m * 128 : (m + 1) * 128, :], in_=y_sb[:])

            def emit_normalize(pr, jj, ps_out_prev):
                qsj = slice(jj * 512, (jj + 1) * 512)
                ssb = rcp.tile([33, 512], f32, name=f"ssb_{pr}_{jj}", tag="ssb")
                for hh in range(2):
                    nc.vector.tensor_copy(
                        ssb[32 * hh : 32 * hh + 1, :],
                        ps_out_prev[64:65, 512 * hh : 512 * (hh + 1)],
                    )
                rc32 = rcp.tile([33, 512], f32, name=f"rc32_{pr}_{jj}", tag="rc32")
                nc.vector.reciprocal_approx_fast(out=rc32[:], in_=ssb[:])
                rc = rcp.tile([33, 512], f32r, name=f"rc_{pr}_{jj}", tag="rc")
                nc.vector.tensor_copy(rc[:], rc32[:])
                bc = psS.tile([128, 1024], f32, name=f"ps_bc_{pr}_{jj}", tag="s")
                for hh in range(2):
                    nc.tensor.matmul(
                        bc[0:64, 512 * hh : 512 * (hh + 1)],
                        consts_sb[32 * hh : 32 * hh + 1, 128:192],
                        rc[32 * hh : 32 * hh + 1, :],
                        start=True,
                        stop=True,
                        tile_position=(32 * hh, 0),
                    )
                bc_sb = bcp.tile([64, 1024], f32, name=f"bc_sb_{pr}_{jj}", tag="bc")
                nc.vector.tensor_copy(bc_sb[:], bc[0:64, :])
                for hh in range(2):
                    nc.vector.tensor_mul(
                        outt_sb[pr][64 * hh : 64 * hh + 64, qsj],
                        ps_out_prev[0:64, 512 * hh : 512 * (hh + 1)],
                        bc_sb[:, 512 * hh : 512 * (hh + 1)],
                    )

            for pr in range(2):
                pending_norm = None  # (pr, j, ps_out) awaiting lazy normalize
                for j in range(QT):
                    n_i = 4 * j + 4
                    qs = slice(j * 512, (j + 1) * 512)
                    ps_out = psO.tile(
                        [65, 1024], f32, name=f"ps_out_{pr}_{j}", tag="o"
                    )
                    prev_et = None
                    prev_i = -1
                    for i in range(n_i):
                        diag = i >= 4 * j
                        r = i - 4 * j
                        pss = psS.tile(
                            [128, 1024], f32, name=f"ps_s{pr}_{j}_{i}", tag="s"
                        )
                        if diag:
                            nw = 128 * (r + 1)
                            for hh in range(2):
                                nc.tensor.matmul(
                                    pss[:, 512 * hh : 512 * hh + nw],
                                    idbf_sb[:, 0:128],
                                    maskt_sb[:, r * 512 : r * 512 + nw],
                                    start=True,
                                    stop=False,
                                )
                        for hh in range(2):
                            hp = slice(64 * hh, 64 * hh + 64)
                            nc.tensor.matmul(
                                pss[:, 512 * hh : 512 * (hh + 1)],
                                kt_sb[pr][hp, i * 128 : (i + 1) * 128],
                                qt_sb[pr][hp, qs],
                                start=not diag,
                                stop=True,
                            )
                        et = etp.tile(
                            [128, 1024], att_dt, name=f"et{pr}_{j}_{i}", tag="et"
                        )
                        nc.scalar.activation(et[:], pss[:], Exp, scale=0.125)
                        if prev_et is not None:
                            for hh in range(2):
                                nc.tensor.matmul(
                                    ps_out[:, 512 * hh : 512 * (hh + 1)],
                                    v_sb[prev_i][:, (2 * pr + hh) * 65 : (2 * pr + hh + 1) * 65],
                                    prev_et[:, 512 * hh : 512 * (hh + 1)],
                                    start=(prev_i == 0),
                                    stop=(prev_i == n_i - 1),
                                )
                        prev_et, prev_i = et, i
                        if i == 1 and pending_norm is not None:
                            emit_normalize(*pending_norm)
                            pending_norm = None
                        if pr == 1 and j > 0 and i == 2:
                            for m in range(4 * (j - 1), 4 * j):
                                emit_outproj_mtile(m)
                    for hh in range(2):
                        nc.tensor.matmul(
                            ps_out[:, 512 * hh : 512 * (hh + 1)],
                            v_sb[n_i - 1][:, (2 * pr + hh) * 65 : (2 * pr + hh + 1) * 65],
                            prev_et[:, 512 * hh : 512 * (hh + 1)],
                            start=(n_i - 1 == 0),
                            stop=True,
                        )
                    pending_norm = (pr, j, ps_out)
                emit_normalize(*pending_norm)
            for m in range(4 * (QT - 1), S // 128):
                emit_outproj_mtile(m)

    nc.compile()
    return nc


def _get_program():
    if "nc" not in _PROG_CACHE:
        _PROG_CACHE["nc"] = _build_program()
    return _PROG_CACHE["nc"]


def _host_prep(query, key, value, mask, w_q, w_k, w_v, w_o):
    query = np.asarray(query, dtype=np.float32)
    key = np.asarray(key, dtype=np.float32)
    value = np.asarray(value, dtype=np.float32)
    w_q = np.asarray(w_q, dtype=np.float32)
    w_k = np.asarray(w_k, dtype=np.float32)
    w_v = np.asarray(w_v, dtype=np.float32)
    w_o = np.asarray(w_o, dtype=np.float32)
    m = np.asarray(mask).reshape(S, S).astype(bool)

    # The kernel's block-skip structure assumes the standard causal mask.
    expected = np.triu(np.ones((S, S), dtype=bool), k=1)
    if not np.array_equal(m, expected):
        raise NotImplementedError("kernel specialized for causal (triu, k=1) mask")

    # 4 canonical diagonal-straddle mask tiles: pattern r covers k-tile
    # 4j+r vs q-tile j; masked where (128r + row) > col.
    import ml_dtypes

    maskt = np.zeros((128, 2048), dtype=np.float32)
    rows = np.arange(128)[:, None]
    cols = np.arange(512)[None, :]
    for r in range(4):
        maskt[:, r * 512 : (r + 1) * 512] = np.where(
            (128 * r + rows) > cols, np.float32(-1e9), np.float32(0.0)
        )
    maskt = maskt.astype(ml_dtypes.bfloat16)
    idbf = np.zeros((128, 132), dtype=ml_dtypes.bfloat16)
    idbf[:, 0:128] = np.eye(128, dtype=ml_dtypes.bfloat16)
    idbf[:, 128:132] = ml_dtypes.bfloat16(1.0)

    consts = np.zeros((128, 193), dtype=np.float32)
    consts[:, 0:128] = np.eye(128, dtype=np.float32)
    consts[:, 128:193] = 1.0

    xt = {}
    for b in range(B):
        xt[("q", b)] = np.ascontiguousarray(query[b].T)
        xt[("k", b)] = np.ascontiguousarray(key[b].T)
        xt[("v", b)] = np.ascontiguousarray(value[b].T)

    in_maps = []
    for c in range(N_CORES):
        b = c // 4
        hb = (c % 4) * HPC
        rs = slice(hb * D_K, (hb + HPC) * D_K)
        in_maps.append(
            {
                "xq": xt[("q", b)],
                "xk": xt[("k", b)],
                "xv": xt[("v", b)],
                "wq": np.ascontiguousarray(w_q[rs, :].T),
                "wk": np.ascontiguousarray(w_k[rs, :].T),
                "wv": np.ascontiguousarray(w_v[rs, :].T),
                "wo": np.ascontiguousarray(w_o[:, rs].T),
                "maskt": maskt,
                "idbf": idbf,
                "consts": consts,
            }
        )
    return in_maps


def kernel(query, key, value, mask, w_q, w_k, w_v, w_o):
    from concourse.bass_utils import run_bass_kernel_spmd

    in_maps = _host_prep(query, key, value, mask, w_q, w_k, w_v, w_o)
    nc = _get_program()
    res = run_bass_kernel_spmd(nc, in_maps, list(range(N_CORES)))
    out = np.zeros((B, S, D_MODEL), dtype=np.float32)
    for c in range(N_CORES):
        out[c // 4] += res.results[c]["y"]
    return out


# revision 19
# speedup vs baseline: 1.0961x; 1.0961x over previous
"""Multi-head causal attention (B=2, S=2048, D=1024, H=16) on 8 TRN2 NeuronCores.

Sharding: batch*head parallel. Core c handles batch b = c//4 and the 4
heads h in [4*(c%4), 4*(c%4)+4). Each core computes its heads' Q/K/V
projections (column-parallel), causal softmax attention, and its partial
row-parallel output projection; the host sums the 4 partial outputs per
batch (the AllReduce of row-parallel tensor parallelism).

On-device layout: everything is kept "transposed" (feature-major) so
every matmul contracts along the partition dimension:
  scoresT[k,q] = K Q^T      (per head, 128-row k-tiles x 512-col q-tiles)
  P^T = exp(scoresT/8 + mask/8)   (additive -1e9 causal mask, PE-accumulated)
  outT[d,q]   = sum_k V[k,d] P^T[k,q]   (PSUM-accumulated over k-tiles)
  sums[q]     = sum_k P^T[k,q]          (ones-vector matmul, col-packed)
  y[q,e]     += sum_hd outT_norm[hd,q] * w_oT[hd,e]
Softmax skips the max-subtraction: scores ~ N(0,1), so exp never
overflows fp32, and exp(-1e9/8) underflows to exactly 0 like the
reference's masked_fill(-1e9).

Matmuls run as float32r (TF32-like, 1 cycle/row at N>=512; measured
~1.5e-4 rms per matmul). Fully-masked 128x512 blocks are skipped
(causal => ~62% of blocks computed).
"""

import numpy as np

D_MODEL = 1024
N_HEADS = 16
D_K = 64
B, S = 2, 2048
N_CORES = 8
HPC = 4            # heads per core
KT = S // 128      # 16 k-tiles
QT = S // 512      # 4 q-tiles
ET = D_MODEL // 128  # 8 e-tiles (contraction tiles for projections)

ATT_BF16 = False  # bf16 scores/attnV matmuls (f32r projections + output proj)

_PROG_CACHE = {}


def _build_program():
    import concourse.bacc as bacc_mod
    import concourse.mybir as mybir
    import concourse.tile as tile

    f32 = mybir.dt.float32
    f32r = mybir.dt.float32r
    bf16 = mybir.dt.bfloat16
    att_dt = bf16 if ATT_BF16 else f32r
    Exp = mybir.ActivationFunctionType.Exp

    nc = bacc_mod.Bacc(
        "TRN2", target_bir_lowering=False, debug=False, num_devices=N_CORES
    )

    xq = nc.dram_tensor("xq", [D_MODEL, S], f32r, kind="ExternalInput").ap()
    xk = nc.dram_tensor("xk", [D_MODEL, S], f32r, kind="ExternalInput").ap()
    xv = nc.dram_tensor("xv", [D_MODEL, S], f32r, kind="ExternalInput").ap()
    wq = nc.dram_tensor("wq", [D_MODEL, 256], f32r, kind="ExternalInput").ap()
    wk = nc.dram_tensor("wk", [D_MODEL, 256], f32r, kind="ExternalInput").ap()
    wv = nc.dram_tensor("wv", [D_MODEL, 256], f32r, kind="ExternalInput").ap()
    wo = nc.dram_tensor("wo", [256, D_MODEL], f32r, kind="ExternalInput").ap()
    maskt = nc.dram_tensor("maskt", [128, 2048], mybir.dt.bfloat16, kind="ExternalInput").ap()
    idbf = nc.dram_tensor("idbf", [128, 132], mybir.dt.bfloat16, kind="ExternalInput").ap()
    consts = nc.dram_tensor("consts", [128, 193], f32r, kind="ExternalInput").ap()
    y = nc.dram_tensor("y", [S, D_MODEL], f32, kind="ExternalOutput").ap()

    with (
        tile.TileContext(nc) as tc,
        nc.allow_low_precision("fp32r attention"),
        tc.tile_pool(name="persist", bufs=1) as pp,
    ):
        # ---- persistent SBUF tiles ----
        def persist(shape, dtype, name):
            return pp.tile(shape, dtype, name=name, tag=name)

        wq_sb = persist([128, ET * 256], f32r, "wq_sb")
        wk_sb = persist([128, ET * 256], f32r, "wk_sb")
        wv_sb = persist([128, ET * 256], f32r, "wv_sb")
        wo_sb = [persist([128, D_MODEL], f32r, f"wo_sb{p}") for p in range(2)]
        maskt_sb = persist([128, 2048], mybir.dt.bfloat16, "maskt_sb")
        idbf_sb = persist([128, 132], mybir.dt.bfloat16, "idbf_sb")
        consts_sb = persist([128, 193], f32r, "consts_sb")
        qt_sb = [persist([128, S], att_dt, f"qt_sb{p}") for p in range(2)]
        kt_sb = [persist([128, S], att_dt, f"kt_sb{p}") for p in range(2)]
        v_sb = [persist([128, 260], att_dt, f"v_sb{i}") for i in range(KT)]
        outt_sb = [persist([128, S], f32r, f"outt_sb{p}") for p in range(2)]

        identity = consts_sb[:, 0:128]
        ones_col = consts_sb[:, 128:192]   # [128, 64] of 1.0
        ones1 = consts_sb[:, 192:193]      # [128, 1] of 1.0

        # weight loads: [1024, 256] -> [128, 8*256] (e-tile t at cols 256t)
        for w_dram, w_tile in ((wq, wq_sb), (wk, wk_sb), (wv, wv_sb)):
            nc.sync.dma_start(
                out=w_tile[:].rearrange("p (t d) -> p t d", t=ET),
                in_=w_dram.rearrange("(t p) d -> p t d", p=128),
            )
        for p in range(2):
            nc.sync.dma_start(out=wo_sb[p][:], in_=wo[p * 128 : (p + 1) * 128, :])
        nc.sync.dma_start(out=maskt_sb[:], in_=maskt[:])
        nc.sync.dma_start(out=idbf_sb[:], in_=idbf[:])
        nc.sync.dma_start(out=consts_sb[:], in_=consts[:])

        # ---- PE warm-up ----
        # The PE HAM clock gate starts (and re-enters) K=4/8 half-clock and
        # only returns to full clock after ~3.4us of gapless PE activity.
        # Dense same-stationary dummy matmuls (results never read) force the
        # transition; packs are re-issued wherever the schedule has an
        # unavoidable multi-us PE idle (DMA-bound ramp, phase boundaries,
        # softmax-normalize tails).
        def emit_warm_pack(pool, count, tag="warm", name="warm"):
            wt = pool.tile([128, 512], f32, name=name, tag=tag)
            for w in range(count):
                nc.tensor.matmul(
                    wt[:],
                    idbf_sb[:, 0:128],
                    maskt_sb[:, 0:512],
                    start=True,
                    stop=True,
                )

        with tc.tile_pool(name="psW", bufs=1, space="PSUM") as psW:
            emit_warm_pack(psW, 40, name="warm_start")

        # ---- phase B: projections ----
        # Q^T/K^T accumulate over all 8 e-tiles into [128, 2048] PSUM (8
        # banks, both m-tiles). The strided xv DMAs are emitted interleaved
        # with the xq/xk streams so the V-projection (which must wait for
        # the QK PSUM banks anyway) starts with its data already resident
        # and runs as a dense PE burst instead of being DMA-paced.
        with (
            tc.tile_pool(name="xe", bufs=3) as xep,
            tc.tile_pool(name="xvk", bufs=10) as xvkp,
        ):
            vdma_tiles = []

            def emit_v_dma():
                i = len(vdma_tiles)
                xvk = xvkp.tile([128, ET * 128], f32r, name=f"xvk_{i}", tag="xvk")
                nc.sync.dma_start(
                    out=xvk[:].rearrange("p (t k) -> p t k", t=ET),
                    in_=xv[:, i * 128 : (i + 1) * 128].rearrange(
                        "(t p) k -> p t k", p=128
                    ),
                )
                vdma_tiles.append(xvk)

            psA_ctx = tc.tile_pool(name="psA", bufs=1, space="PSUM")
            psA = psA_ctx.__enter__()
            for ti, (x_dram, w_tile, dst) in enumerate(
                ((xq, wq_sb, qt_sb), (xk, wk_sb, kt_sb))
            ):
                ps = [
                    psA.tile(
                        [128, S], f32, name=f"ps_p{ti}_{m}", tag=f"proj{m}", bufs=1
                    )
                    for m in range(2)
                ]
                for e in range(ET):
                    xe = xep.tile([128, S], f32r, name=f"xe_{ti}_{e}", tag="xe")
                    nc.sync.dma_start(out=xe[:], in_=x_dram[e * 128 : (e + 1) * 128, :])
                    if ti == 1 or e >= 6:
                        emit_v_dma()
                    for m in range(2):
                        lhsT = w_tile[:, e * 256 + m * 128 : e * 256 + (m + 1) * 128]
                        for n in range(QT):
                            nc.tensor.matmul(
                                ps[m][:, n * 512 : (n + 1) * 512],
                                lhsT,
                                xe[:, n * 512 : (n + 1) * 512],
                                start=(e == 0),
                                stop=(e == ET - 1),
                            )
                for m in range(2):
                    nc.vector.tensor_copy(dst[m][:], ps[m][:])

            psA_ctx.__exit__(None, None, None)
            psV_ctx = tc.tile_pool(name="psV", bufs=2, space="PSUM")
            psV = psV_ctx.__enter__()
            # V projection: dense burst (data already largely resident)
            for i in range(KT):
                if i >= len(vdma_tiles) - 2 and len(vdma_tiles) < KT:
                    emit_v_dma()
                psv = psV.tile([128, 256], f32, name=f"psv_{i}", tag="v")
                xvk = vdma_tiles[i]
                for e in range(ET):
                    nc.tensor.matmul(
                        psv[:],
                        xvk[:, e * 128 : (e + 1) * 128],
                        wv_sb[:, e * 256 : (e + 1) * 256],
                        start=(e == 0),
                        stop=(e == ET - 1),
                    )
                nc.vector.tensor_copy(
                    v_sb[i][:].rearrange("p (h c) -> p h c", c=65)[:, :, 0:64],
                    psv[:].rearrange("p (h d) -> p h d", d=64),
                )
                ones4 = idbf_sb[:, 128:132] if ATT_BF16 else consts_sb[:, 128:132]
                nc.vector.tensor_copy(
                    v_sb[i][:].rearrange("p (h c) -> p h c", c=65)[:, :, 64:65],
                    ones4.rearrange("p (h c) -> p h c", c=1),
                )
            while len(vdma_tiles) < KT:
                emit_v_dma()
            psV_ctx.__exit__(None, None, None)

        # ---- phase C+D: attention with interleaved output projection ----
        # One head-pair per pass (pr = 0, 1). Per (pr, j): score tiles are
        # [128, 1024] head-pair PSUM tiles (row-packed score MMs fill the two
        # banks concurrently; ONE exp per round at FD=1024 runs ~2x faster
        # per element). attnV accumulates into a [65, 1024] pair tile (row
        # 64 = sum of exp via the ones column of v_sb). Normalization of
        # q-block j-1 is emitted lazily inside block j so its DVE chain and
        # broadcast matmuls never stall the PE; the output projection of
        # block j-1 runs as dense filler inside the pr=1 pass.
        with (
            tc.tile_pool(name="psS", bufs=2, space="PSUM") as psS,
            tc.tile_pool(name="psO", bufs=2, space="PSUM") as psO,
            tc.tile_pool(name="et", bufs=6) as etp,
            tc.tile_pool(name="bcsb", bufs=3) as bcp,
            tc.tile_pool(name="rcsb", bufs=3) as rcp,
            tc.tile_pool(name="ysb", bufs=3) as ysbp,
        ):
            def emit_outproj_mtile(m):
                psy = psS.tile([128, 1024], f32, name=f"psy_{m}", tag="s")
                for p in range(2):
                    for n in range(2):
                        nc.tensor.matmul(
                            psy[:, n * 512 : (n + 1) * 512],
                            outt_sb[p][:, m * 128 : (m + 1) * 128],
                            wo_sb[p][:, n * 512 : (n + 1) * 512],
                            start=(p == 0),
                            stop=(p == 1),
                        )
                y_sb = ysbp.tile([128, 1024], f32, name=f"y_sb_{m}", tag="ysb")
                nc.vector.tensor_copy(y_sb[:], psy[:])
                nc.sync.dma_start(out=y[m * 128 : (m + 1) * 128, :], in_=y_sb[:])

            def emit_normalize(pr, jj, ps_out_prev):
                qsj = slice(jj * 512, (jj + 1) * 512)
                ssb = rcp.tile([33, 512], f32, name=f"ssb_{pr}_{jj}", tag="ssb")
                for hh in range(2):
                    nc.vector.tensor_copy(
                        ssb[32 * hh : 32 * hh + 1, :],
                        ps_out_prev[64:65, 512 * hh : 512 * (hh + 1)],
                    )
                rc32 = rcp.tile([33, 512], f32, name=f"rc32_{pr}_{jj}", tag="rc32")
                nc.vector.reciprocal_approx_fast(out=rc32[:], in_=ssb[:])
                rc = rcp.tile([33, 512], f32r, name=f"rc_{pr}_{jj}", tag="rc")
                nc.vector.tensor_copy(rc[:], rc32[:])
                bc = psS.tile([128, 1024], f32, name=f"ps_bc_{pr}_{jj}", tag="s")
                for hh in range(2):
                    nc.tensor.matmul(
                        bc[0:64, 512 * hh : 512 * (hh + 1)],
                        consts_sb[32 * hh : 32 * hh + 1, 128:192],
                        rc[32 * hh : 32 * hh + 1, :],
                        start=True,
                        stop=True,
                        tile_position=(32 * hh, 0),
                    )
                bc_sb = bcp.tile([64, 1024], f32, name=f"bc_sb_{pr}_{jj}", tag="bc")
                nc.vector.tensor_copy(bc_sb[:], bc[0:64, :])
                for hh in range(2):
                    nc.vector.tensor_mul(
                        outt_sb[pr][64 * hh : 64 * hh + 64, qsj],
                        ps_out_prev[0:64, 512 * hh : 512 * (hh + 1)],
                        bc_sb[:, 512 * hh : 512 * (hh + 1)],
                    )

            for pr in range(2):
                pending_norm = None  # (pr, j, ps_out) awaiting lazy normalize
                for j in range(QT):
                    n_i = 4 * j + 4
                    qs = slice(j * 512, (j + 1) * 512)
                    ps_out = psO.tile(
                        [65, 1024], f32, name=f"ps_out_{pr}_{j}", tag="o"
                    )
                    prev_et = None
                    prev_i = -1
                    for i in range(n_i):
                        diag = i >= 4 * j
                        r = i - 4 * j
                        pss = psS.tile(
                            [128, 1024], f32, name=f"ps_s{pr}_{j}_{i}", tag="s"
                        )
                        if diag:
                            nw = 128 * (r + 1)
                            for hh in range(2):
                                nc.tensor.matmul(
                                    pss[:, 512 * hh : 512 * hh + nw],
                                    idbf_sb[:, 0:128],
                                    maskt_sb[:, r * 512 : r * 512 + nw],
                                    start=True,
                                    stop=False,
                                )
                        for hh in range(2):
                            hp = slice(64 * hh, 64 * hh + 64)
                            nc.tensor.matmul(
                                pss[:, 512 * hh : 512 * (hh + 1)],
                                kt_sb[pr][hp, i * 128 : (i + 1) * 128],
                                qt_sb[pr][hp, qs],
                                start=not diag,
                                stop=True,
                            )
                        et = etp.tile(
                            [128, 1024], att_dt, name=f"et{pr}_{j}_{i}", tag="et"
                        )
                        nc.scalar.activation(et[:], pss[:], Exp, scale=0.125)
                        if prev_et is not None:
                            for hh in range(2):
                                nc.tensor.matmul(
                                    ps_out[:, 512 * hh : 512 * (hh + 1)],
                                    v_sb[prev_i][:, (2 * pr + hh) * 65 : (2 * pr + hh + 1) * 65],
                                    prev_et[:, 512 * hh : 512 * (hh + 1)],
                                    start=(prev_i == 0),
                                    stop=(prev_i == n_i - 1),
                                )
                        prev_et, prev_i = et, i
                        if i == 1 and pending_norm is not None:
                            emit_normalize(*pending_norm)
                            pending_norm = None
                        if pr == 1 and j > 0 and i == 2:
                            for m in range(4 * (j - 1), 4 * j):
                                emit_outproj_mtile(m)
                    for hh in range(2):
                        nc.tensor.matmul(
                            ps_out[:, 512 * hh : 512 * (hh + 1)],
                            v_sb[n_i - 1][:, (2 * pr + hh) * 65 : (2 * pr + hh + 1) * 65],
                            prev_et[:, 512 * hh : 512 * (hh + 1)],
                            start=(n_i - 1 == 0),
                            stop=True,
                        )
                    pending_norm = (pr, j, ps_out)
                emit_normalize(*pending_norm)
            for m in range(4 * (QT - 1), S // 128):
                emit_outproj_mtile(m)

    nc.compile()
    return nc


def _get_program():
    if "nc" not in _PROG_CACHE:
        _PROG_CACHE["nc"] = _build_program()
    return _PROG_CACHE["nc"]


def _host_prep(query, key, value, mask, w_q, w_k, w_v, w_o):
    query = np.asarray(query, dtype=np.float32)
    key = np.asarray(key, dtype=np.float32)
    value = np.asarray(value, dtype=np.float32)
    w_q = np.asarray(w_q, dtype=np.float32)
    w_k = np.asarray(w_k, dtype=np.float32)
    w_v = np.asarray(w_v, dtype=np.float32)
    w_o = np.asarray(w_o, dtype=np.float32)
    m = np.asarray(mask).reshape(S, S).astype(bool)

    # The kernel's block-skip structure assumes the standard causal mask.
    expected = np.triu(np.ones((S, S), dtype=bool), k=1)
    if not np.array_equal(m, expected):
        raise NotImplementedError("kernel specialized for causal (triu, k=1) mask")

    # 4 canonical diagonal-straddle mask tiles: pattern r covers k-tile
    # 4j+r vs q-tile j; masked where (128r + row) > col.
    import ml_dtypes

    maskt = np.zeros((128, 2048), dtype=np.float32)
    rows = np.arange(128)[:, None]
    cols = np.arange(512)[None, :]
    for r in range(4):
        maskt[:, r * 512 : (r + 1) * 512] = np.where(
            (128 * r + rows) > cols, np.float32(-1e9), np.float32(0.0)
        )
    maskt = maskt.astype(ml_dtypes.bfloat16)
    idbf = np.zeros((128, 132), dtype=ml_dtypes.bfloat16)
    idbf[:, 0:128] = np.eye(128, dtype=ml_dtypes.bfloat16)
    idbf[:, 128:132] = ml_dtypes.bfloat16(1.0)

    consts = np.zeros((128, 193), dtype=np.float32)
    consts[:, 0:128] = np.eye(128, dtype=np.float32)
    consts[:, 128:193] = 1.0

    xt = {}
    for b in range(B):
        xt[("q", b)] = np.ascontiguousarray(query[b].T)
        xt[("k", b)] = np.ascontiguousarray(key[b].T)
        xt[("v", b)] = np.ascontiguousarray(value[b].T)

    in_maps = []
    for c in range(N_CORES):
        b = c // 4
        hb = (c % 4) * HPC
        rs = slice(hb * D_K, (hb + HPC) * D_K)
        in_maps.append(
            {
                "xq": xt[("q", b)],
                "xk": xt[("k", b)],
                "xv": xt[("v", b)],
                "wq": np.ascontiguousarray(w_q[rs, :].T),
                "wk": np.ascontiguousarray(w_k[rs, :].T),
                "wv": np.ascontiguousarray(w_v[rs, :].T),
                "wo": np.ascontiguousarray(w_o[:, rs].T),
                "maskt": maskt,
                "idbf": idbf,
                "consts": consts,
            }
        )
    return in_maps


def kernel(query, key, value, mask, w_q, w_k, w_v, w_o):
    from concourse.bass_utils import run_bass_kernel_spmd

    in_maps = _host_prep(query, key, value, mask, w_q, w_k, w_v, w_o)
    nc = _get_program()
    res = run_bass_kernel_spmd(nc, in_maps, list(range(N_CORES)))
    out = np.zeros((B, S, D_MODEL), dtype=np.float32)
    for c in range(N_CORES):
        out[c // 4] += res.results[c]["y"]
    return out
